# revision 81
# baseline (speedup 1.0000x reference)
"""Multi-head attention (EMBED=384, 6 heads, S=1024, N=16) on 8 trn2 NeuronCores.

Strategy: data-parallel over batch (2 batches/core), fp16 matmul pipeline.
Per (head, s-tile-of-128-queries) — 48 tiles per batch:
  - 8 score matmuls (lhsT = kT chunk [64,128], rhs = qT s-chunk [64,128])
    fill one 2-bank PSUM tile [128, 1024] laid out (t-chunk, s-local).
  - ONE fat exp [128,1024] -> fp16 probs in SBUF (scale folded in; no
    max-subtraction needed, |scores*scale| < ~6).
  - 8 probs-stationary attn@v matmuls: lhsT = probs chunk [128t, 128s],
    rhs = v65[t-chunk, head] = [128, 64+1] with a ones column, accumulated
    into PSUM [128 s, 65]; col 64 = softmax row-sums.
  - normalization: DVE reciprocal of the sums column + per-partition
    tensor_scalar_mul (also DVE: GPSIMD cannot access PSUM) -> cat fp16.
Per s-tile: 3 PE transposes (via identity) flip cat to catT[f, s]; the output
projection then emits [C, S] f32 tiles DMA'd straight to the output.

Scheduling: a filler deque interleaves the next batch's projections and the
current batch's transposes/out-projections into the attention-tile slots; a
deferred attn@v backlog keeps ACT (the bottleneck at ~100us busy) gapless,
and batch 0 runs a phased tile order (pair-major over the first four s-tiles)
so its own prologue drains at one unit per slot.
"""
import sys

sys.path.insert(0, "/opt/trn_rl_repo")
import numpy as np
import concourse.bass as bass
import concourse.tile as tile
from concourse import mybir
from concourse.bass import ts
from concourse.vector_clock import ScopedClock

f32 = mybir.dt.float32
f16 = mybir.dt.float16

N, C, HW, S = 16, 384, 32, 1024
NH, HD = 6, 64
N3C = 3 * C  # 1152
N_CORES = 8
BPC = N // N_CORES  # batches per core
SCALE = HD**-0.5
MM_DT = f16

# ---------------------------------------------------------------------------
# Workarounds for walrus 1-sync-wait-per-instruction limit
# ---------------------------------------------------------------------------


def _patched_drain_and_barrier(self, tick_clock, wait_clock):
    nc = self.nc
    probe = nc.sync.nop(nofuse=True, hint="drain_waits")
    wait_clock.add_sem_waits(probe.ins, ScopedClock({None: tick_clock.global_clock}))
    inst = probe.ins
    si = inst.sync_info
    waits = list(si.on_wait) if si is not None else []
    if len(waits) > 1:
        inst.sync_info = mybir.SyncInfo(on_wait=[waits[0]], on_update=list(si.on_update))
        for w in waits[1:]:
            extra = nc.sync.nop(nofuse=True, hint="drain_waits")
            extra.ins.sync_info = mybir.SyncInfo(on_wait=[w], on_update=[])
    nc.sync.drain()
    nc.all_engine_barrier()
    assert self.sems is not None
    popped = nc._tile_sem_poison_stack.pop()
    assert popped is self._sem_poison
    nc.clear_and_free_semaphores(list(self.sems.allocated().values()))
    nc.all_engine_barrier()


tile.TileContext._drain_and_barrier = _patched_drain_and_barrier


def _split_multi_waits(nc):
    n_split = 0
    for fn in nc.m.functions:
        for bb in fn.blocks:
            insts = list(bb.instructions)
            out = []
            changed = False
            for inst in insts:
                si = getattr(inst, "sync_info", None)
                try:
                    waits = list(si.on_wait) if si is not None else []
                except Exception:
                    waits = []
                if len(waits) > 1:
                    for w in waits[:-1]:
                        nop = mybir.InstNoOp(name=f"waitsplit-{n_split}")
                        n_split += 1
                        nop.engine = inst.engine
                        nop.sync_info = mybir.SyncInfo(on_wait=[w], on_update=[])
                        out.append(nop)
                    inst.sync_info = mybir.SyncInfo(
                        on_wait=[waits[-1]], on_update=list(si.on_update)
                    )
                    changed = True
                out.append(inst)
            if changed:
                bb.instructions = out
    return n_split


# ---------------------------------------------------------------------------
# Kernel build
# ---------------------------------------------------------------------------


def _build(iters=1):
    nc = bass.Bass("TRN2", target_bir_lowering=False, debug=False, num_devices=N_CORES)
    xs = nc.declare_dram_parameter("xs", [BPC, C, S], MM_DT, isOutput=False)
    wqkvT_d = nc.declare_dram_parameter("wqkvT", [C, N3C], MM_DT, isOutput=False)
    woutT_d = nc.declare_dram_parameter("woutT", [C, C], MM_DT, isOutput=False)
    bout_d = nc.declare_dram_parameter("bout", [C], f32, isOutput=False)
    eye_d = nc.declare_dram_parameter("eye", [128, 128], MM_DT, isOutput=False)
    out_d = nc.declare_dram_parameter("out", [BPC, C, S], f32, isOutput=True)

    with tile.TileContext(nc) as tc:
        with nc.allow_low_precision(reason="fp16 matmul pipeline"):
            _emit(nc, tc, xs, wqkvT_d, woutT_d, bout_d, eye_d, out_d, iters)
    _split_multi_waits(nc)
    return nc


def _emit(nc, tc, xs, wqkvT_d, woutT_d, bout_d, eye_d, out_d, iters=1):
    """Software-pipelined emission: prologue (x load, qkv projections) of the
    next batch and epilogue (transposes, out-projection) of the current batch
    are queued as filler closures drained one per attention-tile slot, so PE
    and ACT stay busy across batch boundaries."""
    import collections
    import contextlib

    ctx = contextlib.ExitStack()
    consts = ctx.enter_context(tc.tile_pool(name="consts", bufs=1))
    xpool = ctx.enter_context(tc.tile_pool(name="xpool", bufs=2))
    qkpool = ctx.enter_context(tc.tile_pool(name="qkpool", bufs=2))
    vpool = ctx.enter_context(tc.tile_pool(name="vpool", bufs=2))
    probpool = ctx.enter_context(tc.tile_pool(name="probpool", bufs=16))
    catpool = ctx.enter_context(tc.tile_pool(name="catpool", bufs=8))
    cattpool = ctx.enter_context(tc.tile_pool(name="cattpool", bufs=2))
    rcpool = ctx.enter_context(tc.tile_pool(name="rcpool", bufs=8))
    fpool = ctx.enter_context(tc.tile_pool(name="fpool", bufs=4))
    # 8 PSUM banks: 2x [128,1024] score/exp tiles (4), 2x attn@v accumulators
    # (2), 2x shared staging for qk/v/transpose/outproj (2). Bigger exp tiles
    # would amortize ACT's per-op access overhead further but cannot be
    # double-buffered within the remaining banks (measured: pipeline stalls
    # outweigh the ~6us ACT saving).
    ps_exp = ctx.enter_context(tc.tile_pool(name="ps_exp", bufs=2, space="PSUM"))
    ps_av = ctx.enter_context(tc.tile_pool(name="ps_av", bufs=2, space="PSUM"))
    ps_mix = ctx.enter_context(tc.tile_pool(name="ps_mix", bufs=2, space="PSUM"))

    # ---- constants ----
    # wq k-chunk 0 loads first; chunks 1-2 are issued after the x DMAs of
    # batch 0 so the serial DMA pipe delivers (wq_k, x_k) pairs in the order
    # the k-accumulation consumes them.
    wq = consts.tile([128, 3, N3C], MM_DT)  # w_qkv^T   k-tile-major
    wq_r = wqkvT_d.rearrange("(k p) c -> p k c", k=3)
    # columns 0:512 cover q-groups j=0..2 and kT j=3 — everything the first
    # scores tile needs; the rest follows behind batch-0's x on the DMA pipe
    nc.scalar.dma_start(out=wq[:, :, 0:512], in_=wq_r[:, :, 0:512])
    wo = consts.tile([128, 3, C], MM_DT)  # w_out^T
    bo = consts.tile([128, 3], f32)
    eye = consts.tile([128, 128], MM_DT)

    # warm the ACT exp table-set during the startup DMA wait
    actwarm = consts.tile([1, 8], f32)
    nc.vector.memset(actwarm, 0.0)
    nc.scalar.activation(
        out=actwarm, in_=actwarm, func=mybir.ActivationFunctionType.Exp, scale=1.0
    )
    # warm the PE pstate ramp during the same wait: junk matmuls keep the
    # tensor engine continuously busy so the first real projections run at
    # full clock instead of the 1/3.7-rate cold pipeline.
    pewarm = consts.tile([128, 72], MM_DT)
    nc.vector.memset(pewarm, 0.0)

    def _load_late_consts():
        # emitted on the scalar queue right after wq chunks 1-2: HWDGE
        # processes dispatches in order, so these transfers queue behind the
        # startup-critical wq/x set on the serial DMA pipe.
        nc.scalar.dma_start(
            out=wo[:, :, :], in_=woutT_d.rearrange("(k p) c -> p k c", k=3)
        )
        nc.scalar.dma_start(out=bo, in_=bout_d.rearrange("(k p) -> p k", k=3))
        nc.scalar.dma_start(out=eye, in_=eye_d[:, :])

    mm = nc.tensor.matmul
    EXP = mybir.ActivationFunctionType.Exp

    nseq = iters * BPC
    state = {}  # seq -> dict(x, qkT, v65, catT, ...)
    filler = collections.deque()
    pending_pro = collections.Counter()  # seq -> un-run prologue closures
    v_done = collections.Counter()  # seq -> completed v units

    def drain(k=1):
        for _ in range(k):
            if filler:
                filler.popleft()()

    def queue_prologue(seq):
        b = seq % BPC
        st = state.setdefault(seq, {})

        def xload():
            x_sb = xpool.tile([128, 3, S], MM_DT, tag="x", name=f"x_{seq}")
            if seq == 0:
                # x0 races wq-crit to the head of the serial DMA pipe; the
                # qk k-accumulation then chases x1/x2 arrivals
                engs = (nc.sync, nc.gpsimd, nc.sync)
            else:
                engs = (nc.sync, nc.sync, nc.sync)
            for k, eng in zip(range(3), engs):
                eng.dma_start(out=x_sb[:, k, :], in_=xs[b, ts(k, 128), :])
            st["x"] = x_sb
            st["qkT"] = qkpool.tile([128, 6, S], MM_DT, tag="qkT", name=f"qkT_{seq}")
            v65 = vpool.tile([128, 8, NH, HD + 1], MM_DT, tag="v65",
                             name=f"v65_{seq}")
            nc.gpsimd.memset(v65[:, :, :, HD : HD + 1], 1.0)
            st["v65"] = v65

        if seq == 0:
            xload()
        else:
            # filler-positioned: issues ~3 slots before the first qk unit
            # needs the data, without polluting the batch-0 startup DMA pipe
            filler.append(xload)
            filler.append(lambda: None)
            filler.append(lambda: None)

        def qk_half(j, u):
            def f():
                x_sb, qkT = st["x"], st["qkT"]
                pq = ps_mix.tile([128, 512], f32, tag="mix", name=f"pq_{seq}_{j}_{u}")
                for k in range(3):
                    mm(pq, wq[:, k, ts(j, 128)], x_sb[:, k, ts(u, 512)],
                       start=(k == 0), stop=(k == 2))
                if seq == 0 and j == 0 and u == 0:
                    # the first scores tile only needs the s=0 slice of qT;
                    # a split copy on ACT (idle until the first exp) takes it
                    # off the DVE copy chain gating the first scores
                    nc.scalar.copy(out=qkT[:, j, 0:128], in_=pq[:, 0:128])
                    nc.vector.tensor_copy(out=qkT[:, j, 128:512], in_=pq[:, 128:512])
                elif seq == 0 and j == 3 and u == 1:
                    nc.scalar.copy(out=qkT[:, j, ts(u, 512)], in_=pq)
                else:
                    nc.vector.tensor_copy(out=qkT[:, j, ts(u, 512)], in_=pq)
                pending_pro[seq] -= 1

            return f

        def v_group(i):
            def f():
                x_sb, v65 = st["x"], st["v65"]
                pv = ps_mix.tile([128, C], f32, tag="mix", name=f"pv_{seq}_{i}")
                for k in range(3):
                    mm(pv, x_sb[:, k, ts(i, 128)], wq[:, k, 2 * C : N3C],
                       start=(k == 0), stop=(k == 2))
                nc.vector.tensor_copy(
                    out=v65[:, i, :, 0:HD],
                    in_=pv.rearrange("p (h d) -> p h d", h=NH),
                )
                pending_pro[seq] -= 1
                v_done[seq] += 1

            return f

        # kT needs both u-halves before any scores of its pair; qT u=0 covers
        # s-tiles 0-3, u=1 (padded behind dummies, draining into this batch's
        # own early slots) covers s-tiles 4-7.
        if seq == 0:
            # phased batch-0 tile order needs pair 1 by slot 8, pair 2 by
            # slot 16, u=1 q-halves by slot 24 — all satisfied at 1 per slot
            for j, u in ((3, 0), (3, 1), (0, 0), (4, 0), (4, 1), (1, 0)):
                filler.append(qk_half(j, u))
            for i in range(8):
                filler.append(v_group(i))
            for j, u in ((5, 0), (5, 1), (2, 0)):
                filler.append(qk_half(j, u))
        else:
            # later batches drain during the previous batch: dummy padding
            # shifts the late-needed units into this batch's own slots
            for j, u in ((3, 0), (3, 1), (0, 0), (4, 0), (4, 1), (1, 0)):
                filler.append(qk_half(j, u))
            for i in range(8):
                filler.append(v_group(i))
            filler.append(lambda: None)
            filler.append(lambda: None)
            for j, u in ((5, 0), (5, 1), (2, 0)):
                filler.append(qk_half(j, u))
            for _ in range(8):
                filler.append(lambda: None)
        for j, u in ((0, 1), (1, 1), (2, 1)):
            filler.append(qk_half(j, u))
        pending_pro[seq] = 20

    def queue_transpose(seq, s, cat_s):
        st = state[seq]

        def f():
            catT = st["catT"]
            pt = ps_mix.tile([128, 3, 128], MM_DT, tag="mix", name=f"pt_{seq}_{s}")
            for j in range(3):
                nc.tensor.transpose(out=pt[:, j, :], in_=cat_s[:, ts(j, 128)],
                                    identity=eye)
            for j in range(3):
                nc.vector.tensor_copy(out=catT[:, j, ts(s, 128)], in_=pt[:, j, :])

        filler.append(lambda: None)  # give the Pool-side norm a slot of slack
        filler.append(f)

    def queue_outproj(seq, s):
        # one unit per transposed s-tile (F=128): spreads across the batch
        # and keeps the post-last-exp tail chain short.
        b = seq % BPC
        st = state[seq]

        def f():
            catT = st["catT"]
            fin = fpool.tile([128, 3, 128], f32, tag="fin", name=f"fin_{seq}_{s}")
            for j in range(3):
                pf = ps_mix.tile([128, 128], f32, tag="mix",
                                 name=f"pf_{seq}_{j}_{s}")
                for k in range(3):
                    mm(pf, wo[:, k, ts(j, 128)], catT[:, k, ts(s, 128)],
                       start=(k == 0), stop=(k == 2))
                nc.vector.tensor_scalar_add(
                    out=fin[:, j, :], in0=pf, scalar1=bo[:, j : j + 1]
                )
            nc.sync.dma_start(
                out=out_d.rearrange("b (j p) s -> b p j s", j=3)[b, :, :, ts(s, 128)],
                in_=fin,
            )
            if s == 7:
                state.pop(seq, None)

        filler.append(f)

    pending = collections.deque()  # (seq, h, s, probs, cat_s) awaiting attn@v
    tailq_t = collections.deque()  # (pop_idx, seq, s, cat_s) awaiting transpose
    tailq_op = collections.deque()  # (pop_idx, seq, s) awaiting outproj
    pops_done = [0]

    def attnv_and_norm(pseq, h, s, probs, cat_s):
        st = state[pseq]
        v65 = st["v65"]
        pav = ps_av.tile([128, HD + 1], f32, tag="av", name=f"pav_{pseq}_{h}_{s}")
        for t in range(8):
            mm(pav, probs[:, ts(t, 128)], v65[:, t, h, :],
               start=(t == 0), stop=(t == 7))
        rcp = rcpool.tile([128, 1], f32, tag="rcp", name=f"rcp_{pseq}_{h}_{s}")
        nc.vector.reciprocal(out=rcp, in_=pav[:, HD : HD + 1])
        # GPSIMD cannot read PSUM, so the normalize multiply runs on DVE
        nc.vector.tensor_scalar_mul(
            out=cat_s[:, ts(h, HD)], in0=pav[:, 0:HD], scalar1=rcp
        )
        # transpose/outproj of a finished s-tile queue a few pops later, so
        # the norms and DVE-side catT copies stay off the PE's critical
        # path (FIFO with pop-distance spacing; handles any tile order)
        pops_done[0] += 1
        if tailq_t and pops_done[0] - tailq_t[0][0] >= 2:
            _, tseq, tss, tcat = tailq_t.popleft()
            queue_transpose(tseq, tss, tcat)
        if tailq_op and pops_done[0] - tailq_op[0][0] >= 4:
            _, oseq, oss = tailq_op.popleft()
            queue_outproj(oseq, oss)
        if pseq == nseq - 1 and s == 7:
            # kernel tail: transpose each head pair as soon as it is
            # normalized, so the final out-projection's k-accumulation
            # overlaps the last heads' attn@v instead of serializing after
            st = state[pseq]
            if h == 1:
                st["pt7"] = ps_mix.tile([128, 3, 128], MM_DT, tag="mix",
                                        name="pt7")
            if h % 2 == 1:
                g = h // 2
                pt = st["pt7"]
                nc.tensor.transpose(out=pt[:, g, :], in_=cat_s[:, ts(g, 128)],
                                    identity=eye)
                nc.vector.tensor_copy(out=st["catT"][:, g, ts(7, 128)],
                                      in_=pt[:, g, :])
            if h == NH - 1:
                queue_outproj(pseq, 7)
        elif h == NH - 1:
            tailq_t.append((pops_done[0], pseq, s, cat_s))
            tailq_op.append((pops_done[0], pseq, s))

    def pop_attnv(limit=2, keep=2):
        n = 0
        while n < limit and len(pending) >= keep and v_done[pending[0][0]] == 8:
            attnv_and_norm(*pending.popleft())
            n += 1

    def emit_batch(seq):
        st = state[seq]
        qkT = st["qkT"]
        catT = cattpool.tile([128, 3, S], MM_DT, tag="catT", name=f"catT_{seq}")
        st["catT"] = catT

        if seq == 0:
            # batch 0: phased tile order — pair 0 over s=0..3 first, then
            # pair 1, pair 2 — so the prologue units are needed at most one
            # per slot and the drain never doubles up (the double-drained
            # slots otherwise starve ACT during the first ~10 tiles)
            order = [(s, 2 * g + hh) for g in range(3) for s in range(4)
                     for hh in range(2)]
            order += [(s, h) for s in range(4, 8) for h in range(NH)]
        else:
            order = [(s, h) for s in range(8) for h in range(NH)]

        cats = {}
        for idx, (s, h) in enumerate(order):
            if s not in cats:
                cats[s] = catpool.tile([128, C], MM_DT, tag="cat",
                                       name=f"cat_{seq}_{s}")
            cat_s = cats[s]
            g, po = h // 2, (h % 2) * 64
            pe_t = ps_exp.tile([128, S], f32, tag="exp", name=f"pe_{seq}_{h}_{s}")
            for t in range(8):
                mm(pe_t[:, ts(t, 128)],
                   qkT[po : po + 64, 3 + g, ts(t, 128)],
                   qkT[po : po + 64, g, ts(s, 128)],
                   start=True, stop=True, tile_position=(po, 0))
            probs = probpool.tile([128, S], MM_DT, tag="probs",
                                  name=f"probs_{seq}_{h}_{s}")
            nc.scalar.activation(out=probs, in_=pe_t, func=EXP, scale=SCALE)
            pending.append((seq, h, s, probs, cat_s))
            keep = 2 if (seq == nseq - 1 and s >= 6) else 4
            pop_attnv(limit=1 if pending_pro[seq] > 0 else 2, keep=keep)
            drain(1 if seq == 0 else (2 if pending_pro[seq] > 0 else 1))

    # ---------------- the pipeline ----------------
    pw = ps_av.tile([128, HD + 1], f32, tag="av", name="pewarm_ps")
    for i in range(150):
        mm(pw[0:72, :], pewarm, pewarm[:, 0 : HD + 1],
           start=True, stop=True, skip_group_check=True)
    queue_prologue(0)
    nc.scalar.dma_start(out=wq[:, :, 512:N3C], in_=wq_r[:, :, 512:N3C])
    _load_late_consts()
    filler.append(lambda: None)
    drain(3)  # qk (3,0) (3,1) (0,0): enough for the first scores tile
    for seq in range(nseq):
        if seq + 1 < nseq:
            queue_prologue(seq + 1)
        emit_batch(seq)
    pop_attnv(limit=len(pending), keep=1)
    while tailq_t:
        _, tseq, tss, tcat = tailq_t.popleft()
        queue_transpose(tseq, tss, tcat)
    while tailq_op:
        _, oseq, oss = tailq_op.popleft()
        queue_outproj(oseq, oss)
    drain(len(filler))

    ctx.close()


_CACHED = None


def _get_nc():
    global _CACHED
    if _CACHED is None:
        _CACHED = _build()
    return _CACHED


def _in_maps(x, w_qkv, w_out, b_out):
    x = np.asarray(x, dtype=np.float32)
    xs_full = np.ascontiguousarray(x.reshape(N, C, S).astype(np.float16))
    wqkvT = np.ascontiguousarray(np.asarray(w_qkv, np.float32).T.astype(np.float16))
    woutT = np.ascontiguousarray(np.asarray(w_out, np.float32).T.astype(np.float16))
    bout = np.ascontiguousarray(np.asarray(b_out, np.float32))
    eye = np.eye(128, dtype=np.float16)
    return [
        {
            "xs": xs_full[i * BPC : (i + 1) * BPC],
            "wqkvT": wqkvT,
            "woutT": woutT,
            "bout": bout,
            "eye": eye,
        }
        for i in range(N_CORES)
    ]


def kernel(x, w_qkv, w_out, b_out):
    from concourse.bass_utils import run_bass_kernel_spmd

    nc = _get_nc()
    res = run_bass_kernel_spmd(nc, _in_maps(x, w_qkv, w_out, b_out), list(range(N_CORES)))
    out = np.concatenate([res.results[i]["out"] for i in range(N_CORES)], axis=0)
    return out.reshape(N, C, HW, HW)
